# revision 63
# baseline (speedup 1.0000x reference)
"""Distributed GQA attention block for Trainium2 (8 NeuronCores).

Problem: nn_Attention_65927747993826
  x:[2,2048,2048] f32, causal GQA attention, H=32 query heads, G=8 KV groups,
  head_size=64, with q/k/v/out projections and bias.

Sharding (8-way head parallel): core c owns query heads [4c, 4c+4) and KV
group c. Each core computes q/k/v projections for its heads from the full x,
causal flash-attention for its 4 heads, and a partial output projection
through its 256 rows of Wo. The host sums the 8 partial outputs and adds the
bias (a per-feature constant commutes with the partial-sum reduction).

Layouts on chip are feature-major ("transposed"): x^T [E, S] etc., so every
matmul contracts over the partition dim with zero on-chip transposes except
v (PE-transposed). Compute dtype bf16 (f32 accumulate in PSUM).
"""

from contextlib import ExitStack

import numpy as np
import ml_dtypes

import concourse.bass as bass
import concourse.mybir as mybir
import concourse.tile as tile
from concourse import bacc
from concourse.bass import ts, ds
from concourse.bass_utils import run_bass_kernel_spmd
from concourse.masks import make_identity, make_upper_triangular

B, S, E = 2, 2048, 2048
H, G, D = 32, 8, 64
NCORES = 8
HPC = H // NCORES            # query heads per core: 4
FPC = HPC * D                # q features per core: 256
P = 128
KT = E // P                  # 16 contraction tiles over E
NT = S // 512                # 4 token 512-blocks per batch
SCALE = D ** -0.5
F32 = mybir.dt.float32
BF16 = mybir.dt.bfloat16
FA = mybir.ActivationFunctionType
ALU = mybir.AluOpType


def build_nc():
    nc = bacc.Bacc()
    # x_t tiled [B, NT, E, 512]: each (b, n) token-block is a dense 2MB
    # region so the strided per-partition DMA rows stay page-local
    x_t = nc.declare_dram_parameter("x_t", [B, NT, E, 512], BF16, isOutput=False)
    wq = nc.declare_dram_parameter("wq", [E, FPC], BF16, isOutput=False)
    wkv = nc.declare_dram_parameter("wkv", [E, P], BF16, isOutput=False)
    wo = nc.declare_dram_parameter("wo", [FPC, E], BF16, isOutput=False)
    # out tiled [B, KT, NT, 128, 512]: every output DMA is one contiguous
    # 128KB block; the host reassembles
    out = nc.declare_dram_parameter("out", [B, KT, NT, P, 512], BF16, isOutput=True)

    with ExitStack() as ctx:
        tc = ctx.enter_context(tile.TileContext(nc))
        consts = ctx.enter_context(tc.tile_pool(name="consts", bufs=1))
        wpool = ctx.enter_context(tc.tile_pool(name="w", bufs=1))
        xbp = ctx.enter_context(tc.tile_pool(name="xb", bufs=5))
        qkvp = ctx.enter_context(tc.tile_pool(name="qkv", bufs=2))
        ppool = ctx.enter_context(tc.tile_pool(name="probs", bufs=10))
        npool = ctx.enter_context(tc.tile_pool(name="norm", bufs=4))
        opool = ctx.enter_context(tc.tile_pool(name="outsb", bufs=4))
        pp_mm = ctx.enter_context(tc.tile_pool(name="pmm", bufs=2, space="PSUM"))
        pp_sp = ctx.enter_context(tc.tile_pool(name="psp", bufs=2, space="PSUM"))
        pp_acc = ctx.enter_context(tc.tile_pool(name="pacc", bufs=2, space="PSUM"))

        # ---- constants ----
        ident = consts.tile([P, P], BF16)
        make_identity(nc, ident)
        tri = consts.tile([P, P], BF16)  # tri[k, q] = 1 iff q >= k
        make_upper_triangular(nc, tri, val=1.0, diag=True)
        # prime the partition_broadcast gpsimd ucode (first use pays a
        # multi-us library-load); do it while weights are still loading
        warm = consts.tile([P, 8], F32)
        nc.gpsimd.memset(warm[0:1, :], 1.0)
        nc.gpsimd.partition_broadcast(warm, warm[0:1, :])

        # ---- weights (gpsimd DMA queue, parallel to the x loads on sync) ----
        wq_sb = wpool.tile([P, KT, FPC], BF16)
        nc.gpsimd.dma_start(wq_sb, wq.rearrange("(ko p) m -> p ko m", p=P))
        wkv_sb = wpool.tile([P, KT, P], BF16)
        nc.gpsimd.dma_start(wkv_sb, wkv.rearrange("(ko p) m -> p ko m", p=P))
        wo_sb = wpool.tile([P, 2, E], BF16)
        nc.gpsimd.dma_start(wo_sb, wo.rearrange("(ko p) m -> p ko m", p=P))

        state = {}

        def gen_proj(b):
            """q/k/v projections + v transpose for batch b (PE-heavy)."""
            # q4: per-head q^T at partitions 0-63, zeros at 64-127 so the
            # scores matmul contracts over a full 128 partitions (keeps the
            # PE in one tile mode — no mode-switch drains).
            q4 = qkvp.tile([P, HPC, S], BF16, tag="q4")
            nc.gpsimd.memset(q4[64:128, :, :], 0.0)
            # k_pad: k^T rows 0-63, zeros 64-127 (scores lhsT)
            k_pad = qkvp.tile([P, S], BF16, tag="kpad")
            nc.gpsimd.memset(k_pad[64:128, :], 0.0)
            # kv: k^T rows 0-63, v^T rows 64-127 (transpose source for v)
            kvsb = qkvp.tile([P, S], BF16, tag="kv")
            state[b] = (q4, k_pad, kvsb)
            for n in range(NT):
                # two half-tiles with separate DMAs: the first 8 k-tiles of
                # matmul only wait on the first half's DMA. The very first
                # block is DMA'd in quarters so the opening matmul can start
                # as early as possible.
                xh = []
                nchunk = 4 if (b == 0 and n == 0) else 1
                for g in range(2):
                    xb = xbp.tile([P, KT // 2, 512], BF16)
                    for q_ in range(nchunk):
                        nc.sync.dma_start(
                            xb[:, ds(q_ * (8 // nchunk), 8 // nchunk), :],
                            x_t[
                                b, n,
                                ds(1024 * g + q_ * (1024 // nchunk), 1024 // nchunk),
                                :,
                            ].rearrange("(ko p) s -> p ko s", p=P),
                        )
                    xh.append(xb)
                for m in range(3):
                    ps = pp_mm.tile([P, 512], F32, tag="mm")
                    for k in range(KT):
                        lhsT = wq_sb[:, k, ts(m, P)] if m < 2 else wkv_sb[:, k, :]
                        nc.tensor.matmul(
                            ps,
                            lhsT,
                            xh[k // 8][:, k % 8, :],
                            start=(k == 0),
                            stop=(k == KT - 1),
                        )
                    if m < 2:
                        nc.vector.tensor_copy(q4[0:64, 2 * m, ts(n, 512)], ps[0:64, :])
                        nc.vector.tensor_copy(
                            q4[0:64, 2 * m + 1, ts(n, 512)], ps[64:128, :]
                        )
                    else:
                        nc.vector.tensor_copy(kvsb[:, ts(n, 512)], ps)
                        nc.vector.tensor_copy(k_pad[0:64, ts(n, 512)], ps[0:64, :])
                    yield

            # v token-major (+ ones column) via PE transpose
            vsb = qkvp.tile([P, S // P, D + 1], BF16, tag="v")
            nc.gpsimd.memset(vsb[:, :, D : D + 1], 1.0)
            state[b] = (q4, k_pad, kvsb, vsb)
            for st in range(S // P):
                tp = pp_mm.tile([P, P], BF16, tag="mm")
                nc.tensor.transpose(tp, kvsb[:, ts(st, P)], ident)
                nc.vector.tensor_copy(vsb[:, st, 0:D], tp[:, 64:128])
                if st % 4 == 3:
                    yield

        def gen_attention(b):
            """causal attention for batch b (ACT-heavy: exp).

            softmax denominators come for free from the ones-column of v_aug
            (row 64 of each accumulator); normalization is deferred so only a
            batched reciprocal + partition-broadcast chain per q-block runs,
            off the matmul critical path.
            """
            q4, k_pad, kvsb, vsb = state[b]
            # attnsb holds UNNORMALIZED attn until the deferred normalize
            attnsb = qkvp.tile([P, 2, S], BF16, tag="attn")
            state[b] = attnsb
            # descending q-blocks: the deepest normalize chain (qt=3) fires
            # earliest and the final one (qt=0) is the shortest, so the
            # out-projection that trails this batch is never left waiting
            for qt in reversed(range(NT)):
                dq = npool.tile([P, 512], F32, tag="den")
                nc.gpsimd.memset(dq, 1.0)
                nkt = 4 * (qt + 1)
                for h in range(HPC):
                    acc = pp_acc.tile([D + 1, 512], F32, tag="acc")
                    nfull = 4 * qt
                    # 1) diagonal scores first: their exp+mask chains get
                    #    maximum slack before their attnV consumers issue last
                    diag_prs = []
                    for t in range(4):
                        kt = nfull + t
                        off = t * P
                        w_ = 512 - off
                        sp = pp_sp.tile([P, 1024], F32, tag="sp")
                        nc.tensor.matmul(
                            sp[:, ds(off, w_)],
                            k_pad[:, ts(kt, P)],
                            q4[:, h, ds(512 * qt + off, w_)],
                            start=True,
                            stop=True,
                        )
                        pr = ppool.tile([P, 1024], BF16, tag="pr")
                        diag_prs.append(pr)
                        nc.scalar.activation(
                            pr[:, ds(off, w_)], sp[:, ds(off, w_)], FA.Exp, scale=SCALE
                        )
                        nc.vector.tensor_tensor(
                            pr[:, ds(off, P)], pr[:, ds(off, P)], tri, ALU.mult
                        )
                    yield
                    # 2) full (off-diagonal) k-tiles, paired two per PSUM tile
                    #    so one exp ACTIVATE covers 1024 columns; attnV follows
                    #    each pair immediately (no mask on its path)
                    first = True
                    for kp in range(0, nfull, 2):
                        sp = pp_sp.tile([P, 1024], F32, tag="sp")
                        for j in range(2):
                            kt = kp + j
                            nc.tensor.matmul(
                                sp[:, ts(j, 512)],
                                k_pad[:, ts(kt, P)],
                                q4[:, h, ts(qt, 512)],
                                start=True,
                                stop=True,
                            )
                        pr = ppool.tile([P, 1024], BF16, tag="pr")
                        nc.scalar.activation(pr, sp, FA.Exp, scale=SCALE)
                        for j in range(2):
                            nc.tensor.matmul(
                                acc,
                                vsb[:, kp + j, :],
                                pr[:, ts(j, 512)],
                                start=first,
                                stop=False,
                            )
                            first = False
                        if kp % 4 == 2:
                            yield
                    # 3) diagonal attnV last
                    for t in range(4):
                        kt = nfull + t
                        off = t * P
                        w_ = 512 - off
                        nc.tensor.matmul(
                            acc[:, ds(off, w_)],
                            vsb[:, kt, :],
                            diag_prs[t][:, ds(off, w_)],
                            start=first,
                            stop=(t == 3),
                        )
                        first = False
                    yield
                    # evacuate unnormalized attn + denominator row (head h's
                    # denominators land at partition 32h for broadcast later)
                    dst = attnsb[64 * (h % 2) : 64 * (h % 2) + 64, h // 2, ts(qt, 512)]
                    nc.vector.tensor_copy(dst, acc[0:D, :])
                    nc.vector.tensor_copy(dq[32 * h : 32 * h + 1, :], acc[D : D + 1, :])
                    yield
                # deferred softmax normalization for this q-block
                rec = npool.tile([P, 512], F32, tag="rec")
                nc.vector.reciprocal(rec, dq)
                for h in range(HPC):
                    # partition_broadcast only honors base partition 0 on HW;
                    # stage the row down to a base-0 tile first (DVE copy keeps
                    # the DMA queues free of dependent waits)
                    rech = npool.tile([1, 512], F32, tag="rech")
                    nc.vector.tensor_copy(rech, rec[32 * h : 32 * h + 1, :])
                    bc = npool.tile([P, 512], F32, tag="bc")
                    nc.gpsimd.partition_broadcast(bc, rech)
                    r0 = 64 * (h % 2)
                    dst = attnsb[r0 : r0 + 64, h // 2, ts(qt, 512)]
                    nc.vector.tensor_tensor(dst, dst, bc[r0 : r0 + 64, :], ALU.mult)
                yield

        def gen_outproj(b, use_act):
            """partial output projection for batch b (PE-heavy).

            n-outer so the last q-block's softmax-normalize latency is hidden
            behind the first 3 n-blocks' matmuls. use_act alternates the PSUM
            evacuation onto ScalarE only when no attention phase is keeping
            ScalarE saturated with exps.
            """
            attnsb = state[b]
            for n in reversed(range(NT)):
                for m in range(KT):
                    po = pp_mm.tile([P, 512], F32, tag="mm")
                    for kk in range(2):
                        nc.tensor.matmul(
                            po,
                            wo_sb[:, kk, ts(m, P)],
                            attnsb[:, kk, ts(n, 512)],
                            start=(kk == 0),
                            stop=(kk == 1),
                        )
                    osb = opool.tile([P, 512], BF16)
                    if m % 3 == 2 or (use_act and m % 3 == 1):
                        nc.scalar.copy(osb, po)
                    else:
                        nc.vector.tensor_copy(osb, po)
                    nc.sync.dma_start(out[b, m, n, :, :], osb)
                    if m % 4 == 3:
                        yield

        def run_all(gen):
            for _ in gen:
                pass

        def interleave(pairs):
            """pairs: list of [gen, steps_per_round]. Round-robin with ratios
            so the PE-filler generator is spread across the whole phase."""
            pairs = [[g, r] for g, r in pairs]
            while pairs:
                for gr in pairs[:]:
                    try:
                        for _ in range(gr[1]):
                            next(gr[0])
                    except StopIteration:
                        pairs.remove(gr)

        def delayed(gen, k):
            for _ in range(k):
                yield
            yield from gen

        # Pipeline the two batches so PE-heavy projection work fills the
        # PE bubbles of the ACT(exp)-bound attention phases. Out-projections
        # enter a phase early (delayed so their first matmuls trail the
        # q-block normalizes they depend on in the in-order PE stream).
        op0 = gen_outproj(0, False)
        op1 = gen_outproj(1, True)
        run_all(gen_proj(0))
        interleave([(gen_attention(0), 4), (gen_proj(1), 1), (delayed(op0, 12), 1)])
        interleave([(op0, 1), (gen_attention(1), 4), (delayed(op1, 12), 1)])
        run_all(op1)
    return nc


BF = ml_dtypes.bfloat16


def make_in_maps(x, Wq, Wk, Wv, Wo):
    # [B, S, E] -> [B, NT, E, 512] (token-block-tiled, feature-major)
    x_t = np.ascontiguousarray(
        np.transpose(
            np.asarray(x, np.float32).reshape(B, NT, 512, E), (0, 1, 3, 2)
        )
    ).astype(BF)
    Wq = np.asarray(Wq, np.float32)
    Wk = np.asarray(Wk, np.float32)
    Wv = np.asarray(Wv, np.float32)
    Wo = np.asarray(Wo, np.float32)
    in_maps = []
    for c in range(NCORES):
        wq_sh = np.ascontiguousarray(Wq[:, FPC * c : FPC * (c + 1)]).astype(BF)
        wkv_sh = np.concatenate(
            [Wk[:, D * c : D * (c + 1)], Wv[:, D * c : D * (c + 1)]], axis=1
        ).astype(BF)
        wo_sh = np.ascontiguousarray(Wo[FPC * c : FPC * (c + 1), :]).astype(BF)
        in_maps.append({"x_t": x_t, "wq": wq_sh, "wkv": wkv_sh, "wo": wo_sh})
    return in_maps


_NC_CACHE = {}


def get_nc():
    if "nc" not in _NC_CACHE:
        nc = build_nc()
        nc.compile()
        _NC_CACHE["nc"] = nc
    return _NC_CACHE["nc"]


def kernel(x, Wq, Wk, Wv, Wo, bo, mask=None, **_ignored):
    nc = get_nc()
    in_maps = make_in_maps(x, Wq, Wk, Wv, Wo)
    res = run_bass_kernel_spmd(nc, in_maps, list(range(NCORES)))
    total = np.zeros((B, KT, NT, P, 512), np.float32)
    for c in range(NCORES):
        total += np.asarray(res.results[c]["out"], np.float32)
    # [B, KT, NT, 128, 512] -> [B, S, E]: feature = m*128+p, token = n*512+s
    full = np.transpose(total, (0, 2, 4, 1, 3)).reshape(B, S, E)
    full = full + np.asarray(bo, np.float32)[None, None, :]
    return np.ascontiguousarray(full)


# revision 64
# speedup vs baseline: 1.0216x; 1.0216x over previous
"""Distributed GQA attention block for Trainium2 (8 NeuronCores).

Problem: nn_Attention_65927747993826
  x:[2,2048,2048] f32, causal GQA attention, H=32 query heads, G=8 KV groups,
  head_size=64, with q/k/v/out projections and bias.

Sharding (8-way head parallel): core c owns query heads [4c, 4c+4) and KV
group c. Each core computes q/k/v projections for its heads from the full x,
causal flash-attention for its 4 heads, and a partial output projection
through its 256 rows of Wo. The host sums the 8 partial outputs and adds the
bias (a per-feature constant commutes with the partial-sum reduction).

Layouts on chip are feature-major ("transposed"): x^T [E, S] etc., so every
matmul contracts over the partition dim with zero on-chip transposes except
v (PE-transposed). Compute dtype bf16 (f32 accumulate in PSUM).
"""

from contextlib import ExitStack

import numpy as np
import ml_dtypes

import concourse.bass as bass
import concourse.mybir as mybir
import concourse.tile as tile
from concourse import bacc
from concourse.bass import ts, ds
from concourse.bass_utils import run_bass_kernel_spmd
from concourse.masks import make_identity, make_upper_triangular

B, S, E = 2, 2048, 2048
H, G, D = 32, 8, 64
NCORES = 8
HPC = H // NCORES            # query heads per core: 4
FPC = HPC * D                # q features per core: 256
P = 128
KT = E // P                  # 16 contraction tiles over E
NT = S // 512                # 4 token 512-blocks per batch
SCALE = D ** -0.5
F32 = mybir.dt.float32
BF16 = mybir.dt.bfloat16
FA = mybir.ActivationFunctionType
ALU = mybir.AluOpType


def build_nc():
    nc = bacc.Bacc()
    # x_t tiled [B, NT, E, 512]: each (b, n) token-block is a dense 2MB
    # region so the strided per-partition DMA rows stay page-local
    x_t = nc.declare_dram_parameter("x_t", [B, NT, E, 512], BF16, isOutput=False)
    wq = nc.declare_dram_parameter("wq", [E, FPC], BF16, isOutput=False)
    wkv = nc.declare_dram_parameter("wkv", [E, P], BF16, isOutput=False)
    wo = nc.declare_dram_parameter("wo", [FPC, E], BF16, isOutput=False)
    # out tiled [B, KT, NT, 128, 512]: every output DMA is one contiguous
    # 128KB block; the host reassembles
    out = nc.declare_dram_parameter("out", [B, KT, NT, P, 512], BF16, isOutput=True)

    with ExitStack() as ctx:
        tc = ctx.enter_context(tile.TileContext(nc))
        consts = ctx.enter_context(tc.tile_pool(name="consts", bufs=1))
        wpool = ctx.enter_context(tc.tile_pool(name="w", bufs=1))
        xbp = ctx.enter_context(tc.tile_pool(name="xb", bufs=5))
        qkvp = ctx.enter_context(tc.tile_pool(name="qkv", bufs=2))
        ppool = ctx.enter_context(tc.tile_pool(name="probs", bufs=10))
        npool = ctx.enter_context(tc.tile_pool(name="norm", bufs=4))
        opool = ctx.enter_context(tc.tile_pool(name="outsb", bufs=4))
        pp_mm = ctx.enter_context(tc.tile_pool(name="pmm", bufs=2, space="PSUM"))
        pp_sp = ctx.enter_context(tc.tile_pool(name="psp", bufs=2, space="PSUM"))
        pp_acc = ctx.enter_context(tc.tile_pool(name="pacc", bufs=2, space="PSUM"))

        # ---- constants ----
        ident = consts.tile([P, P], BF16)
        make_identity(nc, ident)
        tri = consts.tile([P, P], BF16)  # tri[k, q] = 1 iff q >= k
        make_upper_triangular(nc, tri, val=1.0, diag=True)
        # prime the partition_broadcast gpsimd ucode (first use pays a
        # multi-us library-load); do it while weights are still loading
        warm = consts.tile([P, 8], F32)
        nc.gpsimd.memset(warm[0:1, :], 1.0)
        nc.gpsimd.partition_broadcast(warm, warm[0:1, :])

        # ---- weights (gpsimd DMA queue, parallel to the x loads on sync) ----
        wq_sb = wpool.tile([P, KT, FPC], BF16)
        nc.gpsimd.dma_start(wq_sb, wq.rearrange("(ko p) m -> p ko m", p=P))
        wkv_sb = wpool.tile([P, KT, P], BF16)
        nc.gpsimd.dma_start(wkv_sb, wkv.rearrange("(ko p) m -> p ko m", p=P))
        wo_sb = wpool.tile([P, 2, E], BF16)
        nc.gpsimd.dma_start(wo_sb, wo.rearrange("(ko p) m -> p ko m", p=P))

        state = {}

        def gen_proj(b):
            """q/k/v projections + v transpose for batch b (PE-heavy)."""
            # q4: per-head q^T at partitions 0-63, zeros at 64-127 so the
            # scores matmul contracts over a full 128 partitions (keeps the
            # PE in one tile mode — no mode-switch drains).
            q4 = qkvp.tile([P, HPC, S], BF16, tag="q4")
            nc.gpsimd.memset(q4[64:128, :, :], 0.0)
            # k_pad: k^T rows 0-63, zeros 64-127 (scores lhsT)
            k_pad = qkvp.tile([P, S], BF16, tag="kpad")
            nc.gpsimd.memset(k_pad[64:128, :], 0.0)
            # kv: k^T rows 0-63, v^T rows 64-127 (transpose source for v)
            kvsb = qkvp.tile([P, S], BF16, tag="kv")
            state[b] = (q4, k_pad, kvsb)
            for n in range(NT):
                # two half-tiles with separate DMAs: the first 8 k-tiles of
                # matmul only wait on the first half's DMA. The very first
                # block is DMA'd in quarters so the opening matmul can start
                # as early as possible.
                xh = []
                nchunk = 4 if (b == 0 and n == 0) else 1
                for g in range(2):
                    xb = xbp.tile([P, KT // 2, 512], BF16)
                    for q_ in range(nchunk):
                        nc.sync.dma_start(
                            xb[:, ds(q_ * (8 // nchunk), 8 // nchunk), :],
                            x_t[
                                b, n,
                                ds(1024 * g + q_ * (1024 // nchunk), 1024 // nchunk),
                                :,
                            ].rearrange("(ko p) s -> p ko s", p=P),
                        )
                    xh.append(xb)
                for m in range(3):
                    ps = pp_mm.tile([P, 512], F32, tag="mm")
                    for k in range(KT):
                        lhsT = wq_sb[:, k, ts(m, P)] if m < 2 else wkv_sb[:, k, :]
                        nc.tensor.matmul(
                            ps,
                            lhsT,
                            xh[k // 8][:, k % 8, :],
                            start=(k == 0),
                            stop=(k == KT - 1),
                        )
                    if m < 2:
                        nc.vector.tensor_copy(q4[0:64, 2 * m, ts(n, 512)], ps[0:64, :])
                        nc.vector.tensor_copy(
                            q4[0:64, 2 * m + 1, ts(n, 512)], ps[64:128, :]
                        )
                    else:
                        nc.vector.tensor_copy(kvsb[:, ts(n, 512)], ps)
                        nc.vector.tensor_copy(k_pad[0:64, ts(n, 512)], ps[0:64, :])
                    yield

            # v token-major (+ ones column) via PE transpose
            vsb = qkvp.tile([P, S // P, D + 1], BF16, tag="v")
            nc.gpsimd.memset(vsb[:, :, D : D + 1], 1.0)
            state[b] = (q4, k_pad, kvsb, vsb)
            for st in range(S // P):
                tp = pp_mm.tile([P, P], BF16, tag="mm")
                nc.tensor.transpose(tp, kvsb[:, ts(st, P)], ident)
                nc.vector.tensor_copy(vsb[:, st, 0:D], tp[:, 64:128])
                if st % 4 == 3:
                    yield

        def gen_attention(b):
            """causal attention for batch b (ACT-heavy: exp).

            softmax denominators come for free from the ones-column of v_aug
            (row 64 of each accumulator); normalization is deferred so only a
            batched reciprocal + partition-broadcast chain per q-block runs,
            off the matmul critical path.
            """
            q4, k_pad, kvsb, vsb = state[b]
            # attnsb holds UNNORMALIZED attn until the deferred normalize
            attnsb = qkvp.tile([P, 2, S], BF16, tag="attn")
            state[b] = attnsb
            for qt in range(NT):
                dq = npool.tile([P, 512], F32, tag="den")
                nc.gpsimd.memset(dq, 1.0)
                nkt = 4 * (qt + 1)
                for h in range(HPC):
                    acc = pp_acc.tile([D + 1, 512], F32, tag="acc")
                    nfull = 4 * qt
                    # 1) diagonal scores first: their exp+mask chains get
                    #    maximum slack before their attnV consumers issue last
                    diag_prs = []
                    for t in range(4):
                        kt = nfull + t
                        off = t * P
                        w_ = 512 - off
                        sp = pp_sp.tile([P, 1024], F32, tag="sp")
                        nc.tensor.matmul(
                            sp[:, ds(off, w_)],
                            k_pad[:, ts(kt, P)],
                            q4[:, h, ds(512 * qt + off, w_)],
                            start=True,
                            stop=True,
                        )
                        pr = ppool.tile([P, 1024], BF16, tag="pr")
                        diag_prs.append(pr)
                        nc.scalar.activation(
                            pr[:, ds(off, w_)], sp[:, ds(off, w_)], FA.Exp, scale=SCALE
                        )
                        nc.vector.tensor_tensor(
                            pr[:, ds(off, P)], pr[:, ds(off, P)], tri, ALU.mult
                        )
                    yield
                    # 2) full (off-diagonal) k-tiles, paired two per PSUM tile
                    #    so one exp ACTIVATE covers 1024 columns; attnV follows
                    #    each pair immediately (no mask on its path)
                    first = True
                    for kp in range(0, nfull, 2):
                        sp = pp_sp.tile([P, 1024], F32, tag="sp")
                        for j in range(2):
                            kt = kp + j
                            nc.tensor.matmul(
                                sp[:, ts(j, 512)],
                                k_pad[:, ts(kt, P)],
                                q4[:, h, ts(qt, 512)],
                                start=True,
                                stop=True,
                            )
                        pr = ppool.tile([P, 1024], BF16, tag="pr")
                        nc.scalar.activation(pr, sp, FA.Exp, scale=SCALE)
                        for j in range(2):
                            nc.tensor.matmul(
                                acc,
                                vsb[:, kp + j, :],
                                pr[:, ts(j, 512)],
                                start=first,
                                stop=False,
                            )
                            first = False
                        if kp % 4 == 2:
                            yield
                    # 3) diagonal attnV last
                    for t in range(4):
                        kt = nfull + t
                        off = t * P
                        w_ = 512 - off
                        nc.tensor.matmul(
                            acc[:, ds(off, w_)],
                            vsb[:, kt, :],
                            diag_prs[t][:, ds(off, w_)],
                            start=first,
                            stop=(t == 3),
                        )
                        first = False
                    yield
                    # evacuate unnormalized attn + denominator row (head h's
                    # denominators land at partition 32h for broadcast later)
                    dst = attnsb[64 * (h % 2) : 64 * (h % 2) + 64, h // 2, ts(qt, 512)]
                    nc.vector.tensor_copy(dst, acc[0:D, :])
                    nc.vector.tensor_copy(dq[32 * h : 32 * h + 1, :], acc[D : D + 1, :])
                    yield
                # deferred softmax normalization for this q-block
                rec = npool.tile([P, 512], F32, tag="rec")
                nc.vector.reciprocal(rec, dq)
                for h in range(HPC):
                    # partition_broadcast only honors base partition 0 on HW;
                    # stage the row down to a base-0 tile first (DVE copy keeps
                    # the DMA queues free of dependent waits)
                    rech = npool.tile([1, 512], F32, tag="rech")
                    nc.vector.tensor_copy(rech, rec[32 * h : 32 * h + 1, :])
                    bc = npool.tile([P, 512], F32, tag="bc")
                    nc.gpsimd.partition_broadcast(bc, rech)
                    r0 = 64 * (h % 2)
                    dst = attnsb[r0 : r0 + 64, h // 2, ts(qt, 512)]
                    nc.vector.tensor_tensor(dst, dst, bc[r0 : r0 + 64, :], ALU.mult)
                yield

        def gen_outproj(b, use_act):
            """partial output projection for batch b (PE-heavy).

            n-outer so the last q-block's softmax-normalize latency is hidden
            behind the first 3 n-blocks' matmuls. use_act alternates the PSUM
            evacuation onto ScalarE only when no attention phase is keeping
            ScalarE saturated with exps.
            """
            attnsb = state[b]
            for n in range(NT):
                for m in range(KT):
                    po = pp_mm.tile([P, 512], F32, tag="mm")
                    for kk in range(2):
                        nc.tensor.matmul(
                            po,
                            wo_sb[:, kk, ts(m, P)],
                            attnsb[:, kk, ts(n, 512)],
                            start=(kk == 0),
                            stop=(kk == 1),
                        )
                    osb = opool.tile([P, 512], BF16)
                    if m % 3 == 2 or (use_act and m % 3 == 1):
                        nc.scalar.copy(osb, po)
                    else:
                        nc.vector.tensor_copy(osb, po)
                    nc.sync.dma_start(out[b, m, n, :, :], osb)
                    if m % 4 == 3:
                        yield

        def run_all(gen):
            for _ in gen:
                pass

        def interleave(pairs):
            """pairs: list of [gen, steps_per_round]. Round-robin with ratios
            so the PE-filler generator is spread across the whole phase."""
            pairs = [[g, r] for g, r in pairs]
            while pairs:
                for gr in pairs[:]:
                    try:
                        for _ in range(gr[1]):
                            next(gr[0])
                    except StopIteration:
                        pairs.remove(gr)

        def delayed(gen, k):
            for _ in range(k):
                yield
            yield from gen

        # Pipeline the two batches so PE-heavy projection work fills the
        # PE bubbles of the ACT(exp)-bound attention phases. Out-projections
        # enter a phase early (delayed so their first matmuls trail the
        # q-block normalizes they depend on in the in-order PE stream).
        op0 = gen_outproj(0, False)
        op1 = gen_outproj(1, True)
        run_all(gen_proj(0))
        interleave([(gen_attention(0), 4), (gen_proj(1), 1), (delayed(op0, 12), 1)])
        interleave([(op0, 1), (gen_attention(1), 4), (delayed(op1, 12), 1)])
        run_all(op1)
    return nc


BF = ml_dtypes.bfloat16


def make_in_maps(x, Wq, Wk, Wv, Wo):
    # [B, S, E] -> [B, NT, E, 512] (token-block-tiled, feature-major)
    x_t = np.ascontiguousarray(
        np.transpose(
            np.asarray(x, np.float32).reshape(B, NT, 512, E), (0, 1, 3, 2)
        )
    ).astype(BF)
    Wq = np.asarray(Wq, np.float32)
    Wk = np.asarray(Wk, np.float32)
    Wv = np.asarray(Wv, np.float32)
    Wo = np.asarray(Wo, np.float32)
    in_maps = []
    for c in range(NCORES):
        wq_sh = np.ascontiguousarray(Wq[:, FPC * c : FPC * (c + 1)]).astype(BF)
        wkv_sh = np.concatenate(
            [Wk[:, D * c : D * (c + 1)], Wv[:, D * c : D * (c + 1)]], axis=1
        ).astype(BF)
        wo_sh = np.ascontiguousarray(Wo[FPC * c : FPC * (c + 1), :]).astype(BF)
        in_maps.append({"x_t": x_t, "wq": wq_sh, "wkv": wkv_sh, "wo": wo_sh})
    return in_maps


_NC_CACHE = {}


def get_nc():
    if "nc" not in _NC_CACHE:
        nc = build_nc()
        nc.compile()
        _NC_CACHE["nc"] = nc
    return _NC_CACHE["nc"]


def kernel(x, Wq, Wk, Wv, Wo, bo, mask=None, **_ignored):
    nc = get_nc()
    in_maps = make_in_maps(x, Wq, Wk, Wv, Wo)
    res = run_bass_kernel_spmd(nc, in_maps, list(range(NCORES)))
    total = np.zeros((B, KT, NT, P, 512), np.float32)
    for c in range(NCORES):
        total += np.asarray(res.results[c]["out"], np.float32)
    # [B, KT, NT, 128, 512] -> [B, S, E]: feature = m*128+p, token = n*512+s
    full = np.transpose(total, (0, 2, 4, 1, 3)).reshape(B, S, E)
    full = full + np.asarray(bo, np.float32)[None, None, :]
    return np.ascontiguousarray(full)
